# revision 12
# baseline (speedup 1.0000x reference)
"""Multi-head causal attention (B=4, S=2048, D=1024, H=16) on 8 Trainium2 cores.

Sharding: batch x head-group. Core c handles batch c//2 and head-group c%2
(8 heads = 512 features). wq/wk/wv are split column-wise (in x @ w.T terms),
wo row-wise; each pair of cores produces a partial [S, D] output for its batch
which is reduced on the host (the "all-reduce after the output projection").

Device kernel per core (identical SPMD program, inputs pre-sliced/transposed
and rounded to bf16 on host; all matmuls bf16 with fp32 PSUM accumulation):
  - V computed first as [s, f] with a ones-column per head (the PV matmul then
    also produces softmax row-sums), then Q^T/K^T as [f, s] per feature tile so
    attention for early heads overlaps the remaining projections.
  - Attention per head h, sq-half, key-tile j: scores^T [sk, sq] from a K=64
    matmul (columns left of the diagonal never computed), exp on ScalarE with
    fused 1/8 scale straight from PSUM (no max subtraction: scores ~ N(0,1),
    overflow impossible in fp32), causal triangle handled by a post-exp 0/1
    multiply on the 128-wide diagonal block, PV accumulates out^T [65, 1024]
    in PSUM with the diagonal chunk narrowed to skip masked columns.
  - Normalization batched at the end: one ScalarE Reciprocal over all row-sums,
    partition-broadcast via a DRAM bounce, in-place bf16 multiply per head.
  - Output projection row-parallel over 128-row tiles, PSUM accumulation over
    the 4 local feature tiles.
"""

import ml_dtypes
import numpy as np

import concourse.bass as bass
import concourse.mybir as mybir
import concourse.tile as tile
from concourse import bacc
from concourse.bass_utils import run_bass_kernel_spmd

B, S, D, H, HD = 4, 2048, 1024, 16, 64
NCORES = 8
FL = 512          # local features per core (8 heads)
NH = 8            # local heads per core
ND = D // 128     # 8 contraction tiles
NFT = FL // 128   # 4 local feature tiles
NST = S // 128    # 16 sequence tiles

F32 = mybir.dt.float32
BF16 = mybir.dt.bfloat16
EXP = mybir.ActivationFunctionType.Exp
RECIP = mybir.ActivationFunctionType.Reciprocal

BF = ml_dtypes.bfloat16

_CACHE = {}


def _build():
    nc = bacc.Bacc("TRN2", target_bir_lowering=False, debug=False)

    xT = nc.declare_dram_parameter("xT", [D, S], BF16, isOutput=False)
    wqT = nc.declare_dram_parameter("wqT", [D, FL], BF16, isOutput=False)
    wkT = nc.declare_dram_parameter("wkT", [D, FL], BF16, isOutput=False)
    wvT = nc.declare_dram_parameter("wvT", [D, FL], BF16, isOutput=False)
    woT = nc.declare_dram_parameter("woT", [FL, D], BF16, isOutput=False)
    tri01 = nc.declare_dram_parameter("tri01", [128, 128], BF16, isOutput=False)
    ones8 = nc.declare_dram_parameter("ones8", [128, 8], BF16, isOutput=False)
    out = nc.declare_dram_parameter("out", [S, D], F32, isOutput=True)

    xT_t = xT[:].rearrange("(t p) s -> t p s", p=128)
    wqT_t = wqT[:].rearrange("(t p) f -> t p f", p=128)
    wkT_t = wkT[:].rearrange("(t p) f -> t p f", p=128)
    wvT_t = wvT[:].rearrange("(t p) f -> t p f", p=128)
    woT_t = woT[:].rearrange("(t p) o -> t p o", p=128)
    out_t = out[:].rearrange("(t p) o -> t p o", p=128)

    with tile.TileContext(nc) as tc:
        with (
            tc.tile_pool(name="qt", bufs=NFT) as qt_p,
            tc.tile_pool(name="kt", bufs=NFT) as kt_p,
            tc.tile_pool(name="vt", bufs=NST) as vt_p,
            tc.tile_pool(name="msk", bufs=1) as msk_p,
            tc.tile_pool(name="ru", bufs=2) as ru_p,
            tc.tile_pool(name="ps2", bufs=2, space="PSUM") as ps2,
            tc.tile_pool(name="pso", bufs=2, space="PSUM") as pso,
        ):
            tri_sb = msk_p.tile([128, 128], BF16, tag="msk")
            nc.sync.dma_start(tri_sb[:], tri01[:])

            qt = [qt_p.tile([128, S], BF16, tag="qt", name="qt") for _ in range(NFT)]
            kt = [kt_p.tile([128, S], BF16, tag="kt", name="kt") for _ in range(NFT)]
            vt = [vt_p.tile([128, NH * 65], BF16, tag="vt", name="vt") for _ in range(NST)]
            for st in range(NST):
                v3 = vt[st][:].rearrange("p (h c) -> p h c", c=65)
                nc.sync.dma_start(v3[:, :, 64], ones8[:])

            ru = ru_p.tile([NH, S], F32, tag="ru", name="ru")
            rinv = ru_p.tile([NH, S], BF16, tag="ri", name="ri")
            rs = ru_p.tile([65, 1024], F32, tag="rs", name="rs")

            # ---------------- Phase 1: V first, then Q/K per feature tile ----
            with (
                tc.tile_pool(name="xt", bufs=ND) as xt_p,
                tc.tile_pool(name="wi", bufs=3 * ND) as w_p,
            ):
                wq_sb = [w_p.tile([128, FL], BF16, tag="wi", name="wi") for _ in range(ND)]
                wk_sb = [w_p.tile([128, FL], BF16, tag="wi", name="wi") for _ in range(ND)]
                wv_sb = [w_p.tile([128, FL], BF16, tag="wi", name="wi") for _ in range(ND)]
                x_sb = [xt_p.tile([128, S], BF16, tag="xt", name="xt") for _ in range(ND)]
                for d in range(ND):
                    nc.sync.dma_start(wv_sb[d][:], wvT_t[d])
                    nc.sync.dma_start(x_sb[d][:], xT_t[d])
                    nc.sync.dma_start(wk_sb[d][:], wkT_t[d])
                    nc.sync.dma_start(wq_sb[d][:], wqT_t[d])

                # V: [s, f] = x @ wv_local^T, written with per-head stride 65
                for st in range(NST):
                    ps = ps2.tile([128, 512], F32, tag="ps2")
                    for d in range(ND):
                        nc.tensor.matmul(
                            ps[:],
                            x_sb[d][:, st * 128:(st + 1) * 128],
                            wv_sb[d][:],
                            start=(d == 0),
                            stop=(d == ND - 1),
                        )
                    dstv = vt[st][:].rearrange("p (h c) -> p h c", c=65)[:, :, 0:64]
                    srcv = ps[:].rearrange("p (h c) -> p h c", c=64)
                    nc.vector.tensor_copy(dstv, srcv)

                # K^T then Q^T per feature tile (earliest heads first)
                for ft in range(NFT):
                    for wsb, dst in ((wk_sb, kt), (wq_sb, qt)):
                        for c in range(4):
                            ps = ps2.tile([128, 512], F32, tag="ps2")
                            for d in range(ND):
                                nc.tensor.matmul(
                                    ps[:],
                                    wsb[d][:, ft * 128:(ft + 1) * 128],
                                    x_sb[d][:, c * 512:(c + 1) * 512],
                                    start=(d == 0),
                                    stop=(d == ND - 1),
                                )
                            nc.vector.tensor_copy(
                                dst[ft][:, c * 512:(c + 1) * 512], ps[:]
                            )

            # ---------------- Phase 2+3: attention + projection ----------------
            with (
                tc.tile_pool(name="at", bufs=NFT) as at_p,
                tc.tile_pool(name="pt", bufs=4) as pt_p,
                tc.tile_pool(name="rep", bufs=2) as rep_p,
                tc.tile_pool(name="wo", bufs=NFT) as wo_p,
                tc.tile_pool(name="stg", bufs=2) as stg_p,
                tc.tile_pool(name="dbn", bufs=2, space="DRAM") as dbn_p,
            ):
                wo_sb = [wo_p.tile([128, D], BF16, tag="wo", name="wo") for _ in range(NFT)]
                for ft in range(NFT):
                    nc.sync.dma_start(wo_sb[ft][:], woT_t[ft])

                at = [at_p.tile([128, S], BF16, tag="at", name="at") for _ in range(NFT)]

                for h in range(NH):
                    ft, ro = h // 2, (h % 2) * 64
                    for half in range(2):
                        h0 = 1024 * half
                        outT = pso.tile([65, 1024], F32, tag="pso", name="pso")
                        jmax = 8 * half + 8  # key tiles overlapping this half
                        for j in range(jmax):
                            pstart = max(512 * (j // 4), h0)
                            pw = h0 + 1024 - pstart
                            diag = 128 * j >= pstart  # diagonal block in range
                            off = 128 * j - pstart if diag else 0
                            sc = ps2.tile([128, pw], F32, tag="ps2", name="sc")
                            for k in range(0, pw, 512):
                                klo = max(k, off)
                                nc.tensor.matmul(
                                    sc[:, klo:k + 512],
                                    kt[ft][ro:ro + 64, j * 128:(j + 1) * 128],
                                    qt[ft][ro:ro + 64, pstart + klo:pstart + k + 512],
                                    start=True,
                                    stop=True,
                                )
                            ptile = pt_p.tile([128, pw], BF16, tag="pt", name="pt")
                            nc.scalar.activation(
                                ptile[:, off:pw], sc[:, off:pw], EXP, scale=0.125
                            )
                            if diag:
                                # zero the strictly-upper (masked) triangle
                                nc.vector.tensor_mul(
                                    ptile[:, off:off + 128],
                                    ptile[:, off:off + 128],
                                    tri_sb[:],
                                )
                            for k in range(0, pw, 512):
                                klo = max(k, off)
                                c = (pstart + k) // 512  # global 512-chunk id
                                nc.tensor.matmul(
                                    outT[:, pstart - h0 + klo: pstart - h0 + k + 512],
                                    vt[j][:, h * 65:h * 65 + 65],
                                    ptile[:, klo:k + 512],
                                    start=(j == 0),
                                    stop=(j == min(4 * c + 3, jmax - 1)),
                                )
                        # half epilogue: raw copy + rowsum extract (normalize later)
                        nc.vector.tensor_copy(
                            at[ft][ro:ro + 64, h0:h0 + 1024], outT[0:64, :]
                        )
                        nc.vector.tensor_copy(rs[64:65, :], outT[64:65, :])
                        nc.sync.dma_start(ru[h:h + 1, h0:h0 + 1024], rs[64:65, :])

                # batched normalization: one DVE reciprocal over all heads
                with nc.allow_low_precision(reason="softmax 1/rowsum to bf16"):
                    nc.vector.reciprocal(rinv[:, :], ru[:, :])
                for ft in range(NFT):
                    rep = rep_p.tile([128, S], BF16, tag="rep", name="rep")
                    for sub in range(2):
                        h = 2 * ft + sub
                        bounce = dbn_p.tile([1, S], BF16, tag="dbn", name="dbn")
                        nc.sync.dma_start(bounce[:], rinv[h:h + 1, :])
                        nc.sync.dma_start(
                            rep[64 * sub:64 * sub + 64, :],
                            bounce[:].to_broadcast((64, S)),
                        )
                    nc.vector.tensor_mul(at[ft][:], at[ft][:], rep[:])

                # output projection: out[s, :] = sum_f at[f, s] * woT[f, :]
                for st in range(NST):
                    po = ps2.tile([128, D], F32, tag="ps2", name="po")
                    for oc in range(2):
                        for ft in range(NFT):
                            nc.tensor.matmul(
                                po[:, oc * 512:(oc + 1) * 512],
                                at[ft][:, st * 128:(st + 1) * 128],
                                wo_sb[ft][:, oc * 512:(oc + 1) * 512],
                                start=(ft == 0),
                                stop=(ft == NFT - 1),
                            )
                    so = stg_p.tile([128, D], F32, tag="stg", name="stg")
                    nc.vector.tensor_copy(so[:], po[:])
                    nc.sync.dma_start(out_t[st], so[:])

    nc.compile()
    return nc


def kernel(x, wq, wk, wv, wo, _trace=False):
    x = np.asarray(x, dtype=np.float32)
    wq = np.asarray(wq, dtype=np.float32)
    wk = np.asarray(wk, dtype=np.float32)
    wv = np.asarray(wv, dtype=np.float32)
    wo = np.asarray(wo, dtype=np.float32)

    if "nc" not in _CACHE:
        _CACHE["nc"] = _build()
    nc = _CACHE["nc"]

    r = np.arange(128)
    tri = (r[None, :] >= r[:, None]).astype(BF)  # keep where sq >= sk
    ones = np.ones((128, 8), dtype=BF)
    in_maps = []
    for c in range(NCORES):
        b, g = c // 2, c % 2
        fsl = slice(g * FL, (g + 1) * FL)
        in_maps.append(
            {
                "xT": np.ascontiguousarray(x[b].T).astype(BF),
                "wqT": np.ascontiguousarray(wq[fsl, :].T).astype(BF),
                "wkT": np.ascontiguousarray(wk[fsl, :].T).astype(BF),
                "wvT": np.ascontiguousarray(wv[fsl, :].T).astype(BF),
                "woT": np.ascontiguousarray(wo[:, fsl].T).astype(BF),
                "tri01": tri,
                "ones8": ones,
            }
        )

    res = run_bass_kernel_spmd(nc, in_maps, list(range(NCORES)), trace=_trace)
    outs = res.results
    full = np.empty((B, S, D), dtype=np.float32)
    for b in range(B):
        full[b] = outs[2 * b]["out"] + outs[2 * b + 1]["out"]
    if _trace:
        return full, res
    return full


# revision 13
# speedup vs baseline: 1.0682x; 1.0682x over previous
"""Multi-head causal attention (B=4, S=2048, D=1024, H=16) on 8 Trainium2 cores.

Sharding: batch x head-group. Core c handles batch c//2 and head-group c%2
(8 heads = 512 features). wq/wk/wv are split column-wise (in x @ w.T terms),
wo row-wise; each pair of cores produces a partial [S, D] output for its batch
which is reduced on the host (the "all-reduce after the output projection").

Device kernel per core (identical SPMD program, inputs pre-sliced/transposed
and rounded to bf16 on host; all matmuls bf16 with fp32 PSUM accumulation):
  - V computed first as [s, f] with a ones-column per head (the PV matmul then
    also produces softmax row-sums), then Q^T/K^T as [f, s] per feature tile so
    attention for early heads overlaps the remaining projections.
  - Attention per head h, sq-half, key-tile j: scores^T [sk, sq] from a K=64
    matmul (columns left of the diagonal never computed), exp on ScalarE with
    fused 1/8 scale straight from PSUM (no max subtraction: scores ~ N(0,1),
    overflow impossible in fp32), causal triangle handled by a post-exp 0/1
    multiply on the 128-wide diagonal block, PV accumulates out^T [65, 1024]
    in PSUM with the diagonal chunk narrowed to skip masked columns.
  - Normalization batched at the end: one ScalarE Reciprocal over all row-sums,
    partition-broadcast via a DRAM bounce, in-place bf16 multiply per head.
  - Output projection row-parallel over 128-row tiles, PSUM accumulation over
    the 4 local feature tiles.
"""

import ml_dtypes
import numpy as np

import concourse.bass as bass
import concourse.mybir as mybir
import concourse.tile as tile
from concourse import bacc
from concourse.bass_utils import run_bass_kernel_spmd

B, S, D, H, HD = 4, 2048, 1024, 16, 64
NCORES = 8
FL = 512          # local features per core (8 heads)
NH = 8            # local heads per core
ND = D // 128     # 8 contraction tiles
NFT = FL // 128   # 4 local feature tiles
NST = S // 128    # 16 sequence tiles

F32 = mybir.dt.float32
BF16 = mybir.dt.bfloat16
EXP = mybir.ActivationFunctionType.Exp
RECIP = mybir.ActivationFunctionType.Reciprocal

BF = ml_dtypes.bfloat16

_CACHE = {}


def _build():
    nc = bacc.Bacc("TRN2", target_bir_lowering=False, debug=False)

    xT = nc.declare_dram_parameter("xT", [D, S], BF16, isOutput=False)
    wqT = nc.declare_dram_parameter("wqT", [D, FL], BF16, isOutput=False)
    wkT = nc.declare_dram_parameter("wkT", [D, FL], BF16, isOutput=False)
    wvT = nc.declare_dram_parameter("wvT", [D, FL], BF16, isOutput=False)
    woT = nc.declare_dram_parameter("woT", [FL, D], BF16, isOutput=False)
    tri01 = nc.declare_dram_parameter("tri01", [128, 128], BF16, isOutput=False)
    ones8 = nc.declare_dram_parameter("ones8", [128, 8], BF16, isOutput=False)
    out = nc.declare_dram_parameter("out", [S, D], F32, isOutput=True)

    xT_t = xT[:].rearrange("(t p) s -> t p s", p=128)
    wqT_t = wqT[:].rearrange("(t p) f -> t p f", p=128)
    wkT_t = wkT[:].rearrange("(t p) f -> t p f", p=128)
    wvT_t = wvT[:].rearrange("(t p) f -> t p f", p=128)
    woT_t = woT[:].rearrange("(t p) o -> t p o", p=128)
    out_t = out[:].rearrange("(t p) o -> t p o", p=128)

    with tile.TileContext(nc) as tc:
        with (
            tc.tile_pool(name="qt", bufs=NFT) as qt_p,
            tc.tile_pool(name="kt", bufs=NFT) as kt_p,
            tc.tile_pool(name="vt", bufs=NST) as vt_p,
            tc.tile_pool(name="msk", bufs=1) as msk_p,
            tc.tile_pool(name="ru", bufs=2) as ru_p,
            tc.tile_pool(name="ps2", bufs=3, space="PSUM") as ps2,
            tc.tile_pool(name="pso", bufs=1, space="PSUM") as pso,
        ):
            tri_sb = msk_p.tile([128, 128], BF16, tag="msk")
            nc.sync.dma_start(tri_sb[:], tri01[:])

            qt = [qt_p.tile([128, S], BF16, tag="qt", name="qt") for _ in range(NFT)]
            kt = [kt_p.tile([128, S], BF16, tag="kt", name="kt") for _ in range(NFT)]
            vt = [vt_p.tile([128, NH * 65], BF16, tag="vt", name="vt") for _ in range(NST)]
            for st in range(NST):
                v3 = vt[st][:].rearrange("p (h c) -> p h c", c=65)
                nc.sync.dma_start(v3[:, :, 64], ones8[:])

            ru = ru_p.tile([NH, S], BF16, tag="ru", name="ru")
            rinv = ru_p.tile([NH, S], BF16, tag="ri", name="ri")

            # ---------------- Phase 1: V first, then Q/K per feature tile ----
            with (
                tc.tile_pool(name="xt", bufs=ND) as xt_p,
                tc.tile_pool(name="wi", bufs=3 * ND) as w_p,
            ):
                wq_sb = [w_p.tile([128, FL], BF16, tag="wi", name="wi") for _ in range(ND)]
                wk_sb = [w_p.tile([128, FL], BF16, tag="wi", name="wi") for _ in range(ND)]
                wv_sb = [w_p.tile([128, FL], BF16, tag="wi", name="wi") for _ in range(ND)]
                x_sb = [xt_p.tile([128, S], BF16, tag="xt", name="xt") for _ in range(ND)]
                for d in range(ND):
                    nc.sync.dma_start(wv_sb[d][:], wvT_t[d])
                    nc.sync.dma_start(x_sb[d][:], xT_t[d])
                    nc.sync.dma_start(wk_sb[d][:], wkT_t[d])
                    nc.sync.dma_start(wq_sb[d][:], wqT_t[d])

                # V: [s, f] = x @ wv_local^T, written with per-head stride 65
                for st in range(NST):
                    ps = ps2.tile([128, 512], F32, tag="ps2")
                    for d in range(ND):
                        nc.tensor.matmul(
                            ps[:],
                            x_sb[d][:, st * 128:(st + 1) * 128],
                            wv_sb[d][:],
                            start=(d == 0),
                            stop=(d == ND - 1),
                        )
                    dstv = vt[st][:].rearrange("p (h c) -> p h c", c=65)[:, :, 0:64]
                    srcv = ps[:].rearrange("p (h c) -> p h c", c=64)
                    nc.vector.tensor_copy(dstv, srcv)

                # K^T then Q^T per feature tile (earliest heads first)
                for ft in range(NFT):
                    for wsb, dst in ((wk_sb, kt), (wq_sb, qt)):
                        for c in range(4):
                            ps = ps2.tile([128, 512], F32, tag="ps2")
                            for d in range(ND):
                                nc.tensor.matmul(
                                    ps[:],
                                    wsb[d][:, ft * 128:(ft + 1) * 128],
                                    x_sb[d][:, c * 512:(c + 1) * 512],
                                    start=(d == 0),
                                    stop=(d == ND - 1),
                                )
                            nc.vector.tensor_copy(
                                dst[ft][:, c * 512:(c + 1) * 512], ps[:]
                            )

            # ---------------- Phase 2+3: attention + projection ----------------
            with (
                tc.tile_pool(name="at", bufs=NFT) as at_p,
                tc.tile_pool(name="pt", bufs=4) as pt_p,
                tc.tile_pool(name="rep", bufs=2) as rep_p,
                tc.tile_pool(name="wo", bufs=NFT) as wo_p,
                tc.tile_pool(name="stg", bufs=2) as stg_p,
                tc.tile_pool(name="dbn", bufs=2, space="DRAM") as dbn_p,
            ):
                wo_sb = [wo_p.tile([128, D], BF16, tag="wo", name="wo") for _ in range(NFT)]
                for ft in range(NFT):
                    nc.sync.dma_start(wo_sb[ft][:], woT_t[ft])

                at = [at_p.tile([128, S], BF16, tag="at", name="at") for _ in range(NFT)]

                for h in range(NH):
                    ft, ro = h // 2, (h % 2) * 64
                    for half in range(2):
                        h0 = 1024 * half
                        outT = pso.tile([65, 1024], F32, tag="pso", name="pso")
                        jmax = 8 * half + 8  # key tiles overlapping this half
                        for j in range(jmax):
                            pstart = max(512 * (j // 4), h0)
                            pw = h0 + 1024 - pstart
                            diag = 128 * j >= pstart  # diagonal block in range
                            off = 128 * j - pstart if diag else 0
                            sc = ps2.tile([128, pw], F32, tag="ps2", name="sc")
                            for k in range(0, pw, 512):
                                klo = max(k, off)
                                nc.tensor.matmul(
                                    sc[:, klo:k + 512],
                                    kt[ft][ro:ro + 64, j * 128:(j + 1) * 128],
                                    qt[ft][ro:ro + 64, pstart + klo:pstart + k + 512],
                                    start=True,
                                    stop=True,
                                )
                            ptile = pt_p.tile([128, pw], BF16, tag="pt", name="pt")
                            nc.scalar.activation(
                                ptile[:, off:pw], sc[:, off:pw], EXP, scale=0.125
                            )
                            if diag:
                                # zero the strictly-upper (masked) triangle
                                nc.vector.tensor_mul(
                                    ptile[:, off:off + 128],
                                    ptile[:, off:off + 128],
                                    tri_sb[:],
                                )
                            for k in range(0, pw, 512):
                                klo = max(k, off)
                                c = (pstart + k) // 512  # global 512-chunk id
                                nc.tensor.matmul(
                                    outT[:, pstart - h0 + klo: pstart - h0 + k + 512],
                                    vt[j][:, h * 65:h * 65 + 65],
                                    ptile[:, klo:k + 512],
                                    start=(j == 0),
                                    stop=(j == min(4 * c + 3, jmax - 1)),
                                )
                        # half epilogue: one copy releases PSUM; rest off-chain
                        hstg = ru_p.tile([65, 1024], BF16, tag="hstg", name="hstg", bufs=3)
                        nc.vector.tensor_copy(hstg[:, :], outT[:, :])
                        nc.vector.tensor_copy(
                            at[ft][ro:ro + 64, h0:h0 + 1024], hstg[0:64, :]
                        )
                        nc.sync.dma_start(ru[h:h + 1, h0:h0 + 1024], hstg[64:65, :])

                # batched normalization: one DVE reciprocal over all heads
                with nc.allow_low_precision(reason="softmax 1/rowsum to bf16"):
                    nc.vector.reciprocal(rinv[:, :], ru[:, :])
                for ft in range(NFT):
                    rep = rep_p.tile([128, S], BF16, tag="rep", name="rep")
                    for sub in range(2):
                        h = 2 * ft + sub
                        bounce = dbn_p.tile([1, S], BF16, tag="dbn", name="dbn")
                        nc.sync.dma_start(bounce[:], rinv[h:h + 1, :])
                        nc.sync.dma_start(
                            rep[64 * sub:64 * sub + 64, :],
                            bounce[:].to_broadcast((64, S)),
                        )
                    nc.vector.tensor_mul(at[ft][:], at[ft][:], rep[:])

                # output projection: out[s, :] = sum_f at[f, s] * woT[f, :]
                for st in range(NST):
                    po = ps2.tile([128, D], F32, tag="ps2", name="po")
                    for oc in range(2):
                        for ft in range(NFT):
                            nc.tensor.matmul(
                                po[:, oc * 512:(oc + 1) * 512],
                                at[ft][:, st * 128:(st + 1) * 128],
                                wo_sb[ft][:, oc * 512:(oc + 1) * 512],
                                start=(ft == 0),
                                stop=(ft == NFT - 1),
                            )
                    so = stg_p.tile([128, D], F32, tag="stg", name="stg")
                    nc.vector.tensor_copy(so[:], po[:])
                    nc.sync.dma_start(out_t[st], so[:])

    nc.compile()
    return nc


def kernel(x, wq, wk, wv, wo, _trace=False):
    x = np.asarray(x, dtype=np.float32)
    wq = np.asarray(wq, dtype=np.float32)
    wk = np.asarray(wk, dtype=np.float32)
    wv = np.asarray(wv, dtype=np.float32)
    wo = np.asarray(wo, dtype=np.float32)

    if "nc" not in _CACHE:
        _CACHE["nc"] = _build()
    nc = _CACHE["nc"]

    r = np.arange(128)
    tri = (r[None, :] >= r[:, None]).astype(BF)  # keep where sq >= sk
    ones = np.ones((128, 8), dtype=BF)
    in_maps = []
    for c in range(NCORES):
        b, g = c // 2, c % 2
        fsl = slice(g * FL, (g + 1) * FL)
        in_maps.append(
            {
                "xT": np.ascontiguousarray(x[b].T).astype(BF),
                "wqT": np.ascontiguousarray(wq[fsl, :].T).astype(BF),
                "wkT": np.ascontiguousarray(wk[fsl, :].T).astype(BF),
                "wvT": np.ascontiguousarray(wv[fsl, :].T).astype(BF),
                "woT": np.ascontiguousarray(wo[:, fsl].T).astype(BF),
                "tri01": tri,
                "ones8": ones,
            }
        )

    res = run_bass_kernel_spmd(nc, in_maps, list(range(NCORES)), trace=_trace)
    outs = res.results
    full = np.empty((B, S, D), dtype=np.float32)
    for b in range(B):
        full[b] = outs[2 * b]["out"] + outs[2 * b + 1]["out"]
    if _trace:
        return full, res
    return full


# revision 14
# speedup vs baseline: 1.0912x; 1.0215x over previous
"""Multi-head causal attention (B=4, S=2048, D=1024, H=16) on 8 Trainium2 cores.

Sharding: batch x head-group. Core c handles batch c//2 and head-group c%2
(8 heads = 512 features). wq/wk/wv are split column-wise (in x @ w.T terms),
wo row-wise; each pair of cores produces a partial [S, D] output for its batch
which is reduced on the host (the "all-reduce after the output projection").

Device kernel per core (identical SPMD program, inputs pre-sliced/transposed
and rounded to bf16 on host; all matmuls bf16 with fp32 PSUM accumulation):
  - V computed first as [s, f] with a ones-column per head (the PV matmul then
    also produces softmax row-sums), then Q^T/K^T as [f, s] per feature tile so
    attention for early heads overlaps the remaining projections.
  - Attention per head h, sq-half, key-tile j: scores^T [sk, sq] from a K=64
    matmul (columns left of the diagonal never computed), exp on ScalarE with
    fused 1/8 scale straight from PSUM (no max subtraction: scores ~ N(0,1),
    overflow impossible in fp32), causal triangle handled by a post-exp 0/1
    multiply on the 128-wide diagonal block, PV accumulates out^T [65, 1024]
    in PSUM with the diagonal chunk narrowed to skip masked columns.
  - Normalization batched at the end: one ScalarE Reciprocal over all row-sums,
    partition-broadcast via a DRAM bounce, in-place bf16 multiply per head.
  - Output projection row-parallel over 128-row tiles, PSUM accumulation over
    the 4 local feature tiles.
"""

import ml_dtypes
import numpy as np

import concourse.bass as bass
import concourse.mybir as mybir
import concourse.tile as tile
from concourse import bacc
from concourse.bass_utils import run_bass_kernel_spmd

B, S, D, H, HD = 4, 2048, 1024, 16, 64
NCORES = 8
FL = 512          # local features per core (8 heads)
NH = 8            # local heads per core
ND = D // 128     # 8 contraction tiles
NFT = FL // 128   # 4 local feature tiles
NST = S // 128    # 16 sequence tiles

F32 = mybir.dt.float32
BF16 = mybir.dt.bfloat16
EXP = mybir.ActivationFunctionType.Exp
RECIP = mybir.ActivationFunctionType.Reciprocal

BF = ml_dtypes.bfloat16

_CACHE = {}


def _build():
    nc = bacc.Bacc("TRN2", target_bir_lowering=False, debug=False)

    xT = nc.declare_dram_parameter("xT", [D, S], BF16, isOutput=False)
    wqT = nc.declare_dram_parameter("wqT", [D, FL], BF16, isOutput=False)
    wkT = nc.declare_dram_parameter("wkT", [D, FL], BF16, isOutput=False)
    wvT = nc.declare_dram_parameter("wvT", [D, FL], BF16, isOutput=False)
    woT = nc.declare_dram_parameter("woT", [FL, D], BF16, isOutput=False)
    tri01 = nc.declare_dram_parameter("tri01", [128, 128], BF16, isOutput=False)
    ones8 = nc.declare_dram_parameter("ones8", [128, 8], BF16, isOutput=False)
    out = nc.declare_dram_parameter("out", [S, D], F32, isOutput=True)

    xT_t = xT[:].rearrange("(t p) s -> t p s", p=128)
    wqT_t = wqT[:].rearrange("(t p) f -> t p f", p=128)
    wkT_t = wkT[:].rearrange("(t p) f -> t p f", p=128)
    wvT_t = wvT[:].rearrange("(t p) f -> t p f", p=128)
    woT_t = woT[:].rearrange("(t p) o -> t p o", p=128)
    out_t = out[:].rearrange("(t p) o -> t p o", p=128)

    with tile.TileContext(nc) as tc:
        with (
            tc.tile_pool(name="qt", bufs=NFT) as qt_p,
            tc.tile_pool(name="kt", bufs=NFT) as kt_p,
            tc.tile_pool(name="vt", bufs=NST) as vt_p,
            tc.tile_pool(name="msk", bufs=1) as msk_p,
            tc.tile_pool(name="ru", bufs=2) as ru_p,
            tc.tile_pool(name="ps2", bufs=3, space="PSUM") as ps2,
            tc.tile_pool(name="pso", bufs=1, space="PSUM") as pso,
        ):
            tri_sb = msk_p.tile([128, 128], BF16, tag="msk")
            nc.sync.dma_start(tri_sb[:], tri01[:])

            qt = [qt_p.tile([128, S], BF16, tag="qt", name="qt") for _ in range(NFT)]
            kt = [kt_p.tile([128, S], BF16, tag="kt", name="kt") for _ in range(NFT)]
            vt = [vt_p.tile([128, NH * 65], BF16, tag="vt", name="vt") for _ in range(NST)]
            for st in range(NST):
                v3 = vt[st][:].rearrange("p (h c) -> p h c", c=65)
                nc.sync.dma_start(v3[:, :, 64], ones8[:])

            ru = ru_p.tile([NH, S], BF16, tag="ru", name="ru")
            rinv = ru_p.tile([NH, S], BF16, tag="ri", name="ri")

            # ---------------- Phase 1: V first, then Q/K per feature tile ----
            with (
                tc.tile_pool(name="xt", bufs=ND) as xt_p,
                tc.tile_pool(name="wi", bufs=3 * ND) as w_p,
            ):
                wq_sb = [w_p.tile([128, FL], BF16, tag="wi", name="wi") for _ in range(ND)]
                wk_sb = [w_p.tile([128, FL], BF16, tag="wi", name="wi") for _ in range(ND)]
                wv_sb = [w_p.tile([128, FL], BF16, tag="wi", name="wi") for _ in range(ND)]
                x_sb = [xt_p.tile([128, S], BF16, tag="xt", name="xt") for _ in range(ND)]
                for d in range(ND):
                    nc.sync.dma_start(wv_sb[d][:], wvT_t[d])
                    nc.sync.dma_start(x_sb[d][:], xT_t[d])
                    nc.sync.dma_start(wk_sb[d][:], wkT_t[d])
                    nc.sync.dma_start(wq_sb[d][:], wqT_t[d])

                # V: [s, f] = x @ wv_local^T, written with per-head stride 65
                for st in range(NST):
                    ps = ps2.tile([128, 512], F32, tag="ps2")
                    for d in range(ND):
                        nc.tensor.matmul(
                            ps[:],
                            x_sb[d][:, st * 128:(st + 1) * 128],
                            wv_sb[d][:],
                            start=(d == 0),
                            stop=(d == ND - 1),
                        )
                    dstv = vt[st][:].rearrange("p (h c) -> p h c", c=65)[:, :, 0:64]
                    srcv = ps[:].rearrange("p (h c) -> p h c", c=64)
                    nc.vector.tensor_copy(dstv, srcv)

                # K^T then Q^T per feature tile (earliest heads first)
                for ft in range(NFT):
                    for wsb, dst in ((wk_sb, kt), (wq_sb, qt)):
                        for c in range(4):
                            ps = ps2.tile([128, 512], F32, tag="ps2")
                            for d in range(ND):
                                nc.tensor.matmul(
                                    ps[:],
                                    wsb[d][:, ft * 128:(ft + 1) * 128],
                                    x_sb[d][:, c * 512:(c + 1) * 512],
                                    start=(d == 0),
                                    stop=(d == ND - 1),
                                )
                            nc.vector.tensor_copy(
                                dst[ft][:, c * 512:(c + 1) * 512], ps[:]
                            )

            # ---------------- Phase 2+3: attention + projection ----------------
            with (
                tc.tile_pool(name="at", bufs=NFT) as at_p,
                tc.tile_pool(name="pt", bufs=3) as pt_p,
                tc.tile_pool(name="rep", bufs=2) as rep_p,
                tc.tile_pool(name="wo", bufs=NFT) as wo_p,
                tc.tile_pool(name="stg", bufs=2) as stg_p,
                tc.tile_pool(name="dbn", bufs=2, space="DRAM") as dbn_p,
            ):
                wo_sb = [wo_p.tile([128, D], BF16, tag="wo", name="wo") for _ in range(NFT)]
                for ft in range(NFT):
                    nc.sync.dma_start(wo_sb[ft][:], woT_t[ft])

                at = [at_p.tile([128, S], BF16, tag="at", name="at") for _ in range(NFT)]

                for h in range(NH):
                    ft, ro = h // 2, (h % 2) * 64
                    for half in range(2):
                        h0 = 1024 * half
                        outT = pso.tile([65, 1024], F32, tag="pso", name="pso")
                        jmax = 8 * half + 8  # key tiles overlapping this half
                        for j in range(jmax):
                            pstart = max(512 * (j // 4), h0)
                            pw = h0 + 1024 - pstart
                            diag = 128 * j >= pstart  # diagonal block in range
                            off = 128 * j - pstart if diag else 0
                            sc = ps2.tile([128, pw], F32, tag="ps2", name="sc")
                            for k in range(0, pw, 512):
                                nc.tensor.matmul(
                                    sc[:, k:k + 512],
                                    kt[ft][ro:ro + 64, j * 128:(j + 1) * 128],
                                    qt[ft][ro:ro + 64, pstart + k:pstart + k + 512],
                                    start=True,
                                    stop=True,
                                )
                            ptile = pt_p.tile([128, pw], BF16, tag="pt", name="pt")
                            nc.scalar.activation(
                                ptile[:, off:pw], sc[:, off:pw], EXP, scale=0.125
                            )
                            if diag:
                                # zero the strictly-upper (masked) triangle
                                nc.vector.tensor_mul(
                                    ptile[:, off:off + 128],
                                    ptile[:, off:off + 128],
                                    tri_sb[:],
                                )
                            for k in range(0, pw, 512):
                                klo = max(k, off)
                                c = (pstart + k) // 512  # global 512-chunk id
                                nc.tensor.matmul(
                                    outT[:, pstart - h0 + klo: pstart - h0 + k + 512],
                                    vt[j][:, h * 65:h * 65 + 65],
                                    ptile[:, klo:k + 512],
                                    start=(j == 0),
                                    stop=(j == min(4 * c + 3, jmax - 1)),
                                )
                        # half epilogue: one copy releases PSUM; rest off-chain
                        hstg = ru_p.tile([65, 1024], BF16, tag="hstg", name="hstg", bufs=3)
                        nc.vector.tensor_copy(hstg[:, :], outT[:, :])
                        nc.vector.tensor_copy(
                            at[ft][ro:ro + 64, h0:h0 + 1024], hstg[0:64, :]
                        )
                        nc.sync.dma_start(ru[h:h + 1, h0:h0 + 1024], hstg[64:65, :])

                # batched normalization: one DVE reciprocal over all heads
                with nc.allow_low_precision(reason="softmax 1/rowsum to bf16"):
                    nc.vector.reciprocal(rinv[:, :], ru[:, :])
                for ft in range(NFT):
                    rep = rep_p.tile([128, S], BF16, tag="rep", name="rep")
                    for sub in range(2):
                        h = 2 * ft + sub
                        bounce = dbn_p.tile([1, S], BF16, tag="dbn", name="dbn")
                        nc.sync.dma_start(bounce[:], rinv[h:h + 1, :])
                        nc.sync.dma_start(
                            rep[64 * sub:64 * sub + 64, :],
                            bounce[:].to_broadcast((64, S)),
                        )
                    nc.vector.tensor_mul(at[ft][:], at[ft][:], rep[:])

                # output projection: out[s, :] = sum_f at[f, s] * woT[f, :]
                for st in range(NST):
                    po = ps2.tile([128, D], F32, tag="ps2", name="po")
                    for oc in range(2):
                        for ft in range(NFT):
                            nc.tensor.matmul(
                                po[:, oc * 512:(oc + 1) * 512],
                                at[ft][:, st * 128:(st + 1) * 128],
                                wo_sb[ft][:, oc * 512:(oc + 1) * 512],
                                start=(ft == 0),
                                stop=(ft == NFT - 1),
                            )
                    so = stg_p.tile([128, D], F32, tag="stg", name="stg")
                    nc.vector.tensor_copy(so[:], po[:])
                    nc.sync.dma_start(out_t[st], so[:])

    nc.compile()
    return nc


def kernel(x, wq, wk, wv, wo, _trace=False):
    x = np.asarray(x, dtype=np.float32)
    wq = np.asarray(wq, dtype=np.float32)
    wk = np.asarray(wk, dtype=np.float32)
    wv = np.asarray(wv, dtype=np.float32)
    wo = np.asarray(wo, dtype=np.float32)

    if "nc" not in _CACHE:
        _CACHE["nc"] = _build()
    nc = _CACHE["nc"]

    r = np.arange(128)
    tri = (r[None, :] >= r[:, None]).astype(BF)  # keep where sq >= sk
    ones = np.ones((128, 8), dtype=BF)
    in_maps = []
    for c in range(NCORES):
        b, g = c // 2, c % 2
        fsl = slice(g * FL, (g + 1) * FL)
        in_maps.append(
            {
                "xT": np.ascontiguousarray(x[b].T).astype(BF),
                "wqT": np.ascontiguousarray(wq[fsl, :].T).astype(BF),
                "wkT": np.ascontiguousarray(wk[fsl, :].T).astype(BF),
                "wvT": np.ascontiguousarray(wv[fsl, :].T).astype(BF),
                "woT": np.ascontiguousarray(wo[:, fsl].T).astype(BF),
                "tri01": tri,
                "ones8": ones,
            }
        )

    res = run_bass_kernel_spmd(nc, in_maps, list(range(NCORES)), trace=_trace)
    outs = res.results
    full = np.empty((B, S, D), dtype=np.float32)
    for b in range(B):
        full[b] = outs[2 * b]["out"] + outs[2 * b + 1]["out"]
    if _trace:
        return full, res
    return full


# revision 15
# speedup vs baseline: 1.1333x; 1.0386x over previous
"""Multi-head causal attention (B=4, S=2048, D=1024, H=16) on 8 Trainium2 cores.

Sharding: batch x head-group. Core c handles batch c//2 and head-group c%2
(8 heads = 512 features). wq/wk/wv are split column-wise (in x @ w.T terms),
wo row-wise; each pair of cores produces a partial [S, D] output for its batch
which is reduced on the host (the "all-reduce after the output projection").

Device kernel per core (identical SPMD program, inputs pre-sliced/transposed
and rounded to bf16 on host; all matmuls bf16 with fp32 PSUM accumulation):
  - V computed first as [s, f] with a ones-column per head (the PV matmul then
    also produces softmax row-sums), then Q^T/K^T as [f, s] per feature tile so
    attention for early heads overlaps the remaining projections.
  - Attention per head h, sq-half, key-tile j: scores^T [sk, sq] from a K=64
    matmul (columns left of the diagonal never computed), exp on ScalarE with
    fused 1/8 scale straight from PSUM (no max subtraction: scores ~ N(0,1),
    overflow impossible in fp32), causal triangle handled by a post-exp 0/1
    multiply on the 128-wide diagonal block, PV accumulates out^T [65, 1024]
    in PSUM with the diagonal chunk narrowed to skip masked columns.
  - Normalization batched at the end: one ScalarE Reciprocal over all row-sums,
    partition-broadcast via a DRAM bounce, in-place bf16 multiply per head.
  - Output projection row-parallel over 128-row tiles, PSUM accumulation over
    the 4 local feature tiles.
"""

import ml_dtypes
import numpy as np

import concourse.bass as bass
import concourse.mybir as mybir
import concourse.tile as tile
from concourse import bacc
from concourse.bass_utils import run_bass_kernel_spmd

B, S, D, H, HD = 4, 2048, 1024, 16, 64
NCORES = 8
FL = 512          # local features per core (8 heads)
NH = 8            # local heads per core
ND = D // 128     # 8 contraction tiles
NFT = FL // 128   # 4 local feature tiles
NST = S // 128    # 16 sequence tiles

F32 = mybir.dt.float32
BF16 = mybir.dt.bfloat16
EXP = mybir.ActivationFunctionType.Exp
RECIP = mybir.ActivationFunctionType.Reciprocal

BF = ml_dtypes.bfloat16

_CACHE = {}


def _build():
    nc = bacc.Bacc("TRN2", target_bir_lowering=False, debug=False)

    xT = nc.declare_dram_parameter("xT", [D, S], BF16, isOutput=False)
    wqT = nc.declare_dram_parameter("wqT", [D, FL], BF16, isOutput=False)
    wkT = nc.declare_dram_parameter("wkT", [D, FL], BF16, isOutput=False)
    wvT = nc.declare_dram_parameter("wvT", [D, FL], BF16, isOutput=False)
    woT = nc.declare_dram_parameter("woT", [FL, D], BF16, isOutput=False)
    tri01 = nc.declare_dram_parameter("tri01", [128, 128], BF16, isOutput=False)
    ones8 = nc.declare_dram_parameter("ones8", [128, 8], BF16, isOutput=False)
    out = nc.declare_dram_parameter("out", [S, D], F32, isOutput=True)

    xT_t = xT[:].rearrange("(t p) s -> t p s", p=128)
    wqT_t = wqT[:].rearrange("(t p) f -> t p f", p=128)
    wkT_t = wkT[:].rearrange("(t p) f -> t p f", p=128)
    wvT_t = wvT[:].rearrange("(t p) f -> t p f", p=128)
    woT_t = woT[:].rearrange("(t p) o -> t p o", p=128)
    out_t = out[:].rearrange("(t p) o -> t p o", p=128)

    with tile.TileContext(nc) as tc:
        with (
            tc.tile_pool(name="qt", bufs=NFT) as qt_p,
            tc.tile_pool(name="kt", bufs=NFT) as kt_p,
            tc.tile_pool(name="vt", bufs=NST) as vt_p,
            tc.tile_pool(name="msk", bufs=1) as msk_p,
            tc.tile_pool(name="ru", bufs=2) as ru_p,
            tc.tile_pool(name="ps2", bufs=3, space="PSUM") as ps2,
            tc.tile_pool(name="pso", bufs=1, space="PSUM") as pso,
        ):
            tri_sb = msk_p.tile([128, 128], BF16, tag="msk")
            nc.sync.dma_start(tri_sb[:], tri01[:])

            qt = [qt_p.tile([128, S], BF16, tag="qt", name="qt") for _ in range(NFT)]
            kt = [kt_p.tile([128, S], BF16, tag="kt", name="kt") for _ in range(NFT)]
            vt = [vt_p.tile([128, NH * 65], BF16, tag="vt", name="vt") for _ in range(NST)]
            for st in range(NST):
                v3 = vt[st][:].rearrange("p (h c) -> p h c", c=65)
                nc.sync.dma_start(v3[:, :, 64], ones8[:])

            ru = ru_p.tile([NH, S], F32, tag="ru", name="ru")
            rinv = ru_p.tile([NH, S], BF16, tag="ri", name="ri")
            rs = ru_p.tile([65, 1024], F32, tag="rs", name="rs")

            # ---------------- Phase 1: V first, then Q/K per feature tile ----
            with (
                tc.tile_pool(name="xt", bufs=ND) as xt_p,
                tc.tile_pool(name="wi", bufs=3 * ND) as w_p,
            ):
                wq_sb = [w_p.tile([128, FL], BF16, tag="wi", name="wi") for _ in range(ND)]
                wk_sb = [w_p.tile([128, FL], BF16, tag="wi", name="wi") for _ in range(ND)]
                wv_sb = [w_p.tile([128, FL], BF16, tag="wi", name="wi") for _ in range(ND)]
                x_sb = [xt_p.tile([128, S], BF16, tag="xt", name="xt") for _ in range(ND)]
                for d in range(ND):
                    nc.sync.dma_start(wv_sb[d][:], wvT_t[d])
                    nc.sync.dma_start(x_sb[d][:], xT_t[d])
                    nc.sync.dma_start(wk_sb[d][:], wkT_t[d])
                    nc.sync.dma_start(wq_sb[d][:], wqT_t[d])

                # V: [s, f] = x @ wv_local^T, written with per-head stride 65
                for st in range(NST):
                    ps = ps2.tile([128, 512], F32, tag="ps2")
                    for d in range(ND):
                        nc.tensor.matmul(
                            ps[:],
                            x_sb[d][:, st * 128:(st + 1) * 128],
                            wv_sb[d][:],
                            start=(d == 0),
                            stop=(d == ND - 1),
                        )
                    dstv = vt[st][:].rearrange("p (h c) -> p h c", c=65)[:, :, 0:64]
                    srcv = ps[:].rearrange("p (h c) -> p h c", c=64)
                    nc.vector.tensor_copy(dstv, srcv)

                # K^T then Q^T per feature tile (earliest heads first)
                for ft in range(NFT):
                    for wsb, dst in ((wk_sb, kt), (wq_sb, qt)):
                        for c in range(4):
                            ps = ps2.tile([128, 512], F32, tag="ps2")
                            for d in range(ND):
                                nc.tensor.matmul(
                                    ps[:],
                                    wsb[d][:, ft * 128:(ft + 1) * 128],
                                    x_sb[d][:, c * 512:(c + 1) * 512],
                                    start=(d == 0),
                                    stop=(d == ND - 1),
                                )
                            nc.vector.tensor_copy(
                                dst[ft][:, c * 512:(c + 1) * 512], ps[:]
                            )

            # ---------------- Phase 2+3: attention + projection ----------------
            with (
                tc.tile_pool(name="at", bufs=NFT) as at_p,
                tc.tile_pool(name="pt", bufs=3) as pt_p,
                tc.tile_pool(name="rep", bufs=2) as rep_p,
                tc.tile_pool(name="wo", bufs=NFT) as wo_p,
                tc.tile_pool(name="stg", bufs=2) as stg_p,
                tc.tile_pool(name="dbn", bufs=2, space="DRAM") as dbn_p,
            ):
                wo_sb = [wo_p.tile([128, D], BF16, tag="wo", name="wo") for _ in range(NFT)]
                for ft in range(NFT):
                    nc.sync.dma_start(wo_sb[ft][:], woT_t[ft])

                at = [at_p.tile([128, S], BF16, tag="at", name="at") for _ in range(NFT)]

                for h in range(NH):
                    ft, ro = h // 2, (h % 2) * 64
                    for half in range(2):
                        h0 = 1024 * half
                        outT = pso.tile([65, 1024], F32, tag="pso", name="pso")
                        jmax = 8 * half + 8  # key tiles overlapping this half
                        for j in range(jmax):
                            pstart = max(512 * (j // 4), h0)
                            pw = h0 + 1024 - pstart
                            diag = 128 * j >= pstart  # diagonal block in range
                            off = 128 * j - pstart if diag else 0
                            sc = ps2.tile([128, pw], F32, tag="ps2", name="sc")
                            for k in range(0, pw, 512):
                                nc.tensor.matmul(
                                    sc[:, k:k + 512],
                                    kt[ft][ro:ro + 64, j * 128:(j + 1) * 128],
                                    qt[ft][ro:ro + 64, pstart + k:pstart + k + 512],
                                    start=True,
                                    stop=True,
                                )
                            ptile = pt_p.tile([128, pw], BF16, tag="pt", name="pt")
                            nc.scalar.activation(
                                ptile[:, off:pw], sc[:, off:pw], EXP, scale=0.125
                            )
                            if diag:
                                # zero the strictly-upper (masked) triangle
                                nc.vector.tensor_mul(
                                    ptile[:, off:off + 128],
                                    ptile[:, off:off + 128],
                                    tri_sb[:],
                                )
                            for k in range(0, pw, 512):
                                klo = max(k, off)
                                c = (pstart + k) // 512  # global 512-chunk id
                                nc.tensor.matmul(
                                    outT[:, pstart - h0 + klo: pstart - h0 + k + 512],
                                    vt[j][:, h * 65:h * 65 + 65],
                                    ptile[:, klo:k + 512],
                                    start=(j == 0),
                                    stop=(j == min(4 * c + 3, jmax - 1)),
                                )
                        # half epilogue: raw copy + rowsum extract (normalize later)
                        nc.vector.tensor_copy(
                            at[ft][ro:ro + 64, h0:h0 + 1024], outT[0:64, :]
                        )
                        nc.vector.tensor_copy(rs[64:65, :], outT[64:65, :])
                        nc.sync.dma_start(ru[h:h + 1, h0:h0 + 1024], rs[64:65, :])

                # batched normalization: one DVE reciprocal over all heads
                with nc.allow_low_precision(reason="softmax 1/rowsum to bf16"):
                    nc.vector.reciprocal(rinv[:, :], ru[:, :])
                for ft in range(NFT):
                    rep = rep_p.tile([128, S], BF16, tag="rep", name="rep")
                    for sub in range(2):
                        h = 2 * ft + sub
                        bounce = dbn_p.tile([1, S], BF16, tag="dbn", name="dbn")
                        nc.sync.dma_start(bounce[:], rinv[h:h + 1, :])
                        nc.sync.dma_start(
                            rep[64 * sub:64 * sub + 64, :],
                            bounce[:].to_broadcast((64, S)),
                        )
                    nc.vector.tensor_mul(at[ft][:], at[ft][:], rep[:])

                # output projection: out[s, :] = sum_f at[f, s] * woT[f, :]
                for st in range(NST):
                    po = ps2.tile([128, D], F32, tag="ps2", name="po")
                    for oc in range(2):
                        for ft in range(NFT):
                            nc.tensor.matmul(
                                po[:, oc * 512:(oc + 1) * 512],
                                at[ft][:, st * 128:(st + 1) * 128],
                                wo_sb[ft][:, oc * 512:(oc + 1) * 512],
                                start=(ft == 0),
                                stop=(ft == NFT - 1),
                            )
                    so = stg_p.tile([128, D], F32, tag="stg", name="stg")
                    nc.vector.tensor_copy(so[:], po[:])
                    nc.sync.dma_start(out_t[st], so[:])

    nc.compile()
    return nc


def kernel(x, wq, wk, wv, wo, _trace=False):
    x = np.asarray(x, dtype=np.float32)
    wq = np.asarray(wq, dtype=np.float32)
    wk = np.asarray(wk, dtype=np.float32)
    wv = np.asarray(wv, dtype=np.float32)
    wo = np.asarray(wo, dtype=np.float32)

    if "nc" not in _CACHE:
        _CACHE["nc"] = _build()
    nc = _CACHE["nc"]

    r = np.arange(128)
    tri = (r[None, :] >= r[:, None]).astype(BF)  # keep where sq >= sk
    ones = np.ones((128, 8), dtype=BF)
    in_maps = []
    for c in range(NCORES):
        b, g = c // 2, c % 2
        fsl = slice(g * FL, (g + 1) * FL)
        in_maps.append(
            {
                "xT": np.ascontiguousarray(x[b].T).astype(BF),
                "wqT": np.ascontiguousarray(wq[fsl, :].T).astype(BF),
                "wkT": np.ascontiguousarray(wk[fsl, :].T).astype(BF),
                "wvT": np.ascontiguousarray(wv[fsl, :].T).astype(BF),
                "woT": np.ascontiguousarray(wo[:, fsl].T).astype(BF),
                "tri01": tri,
                "ones8": ones,
            }
        )

    res = run_bass_kernel_spmd(nc, in_maps, list(range(NCORES)), trace=_trace)
    outs = res.results
    full = np.empty((B, S, D), dtype=np.float32)
    for b in range(B):
        full[b] = outs[2 * b]["out"] + outs[2 * b + 1]["out"]
    if _trace:
        return full, res
    return full
